# revision 11
# baseline (speedup 1.0000x reference)
"""Trainium2 Bass kernel for nn_GCNConvNet (MFConv GNN, N=100k, E=1.6M).

Strategy (8 NeuronCores, SPMD):
  - Nodes renumbered on host: dealt round-robin per degree-bucket so every
    core owns R rows laid out bucket-contiguously (uniform bucket offsets
    across cores -> one shared program). Pad rows stay exactly zero.
  - Activations live row-major bf16 in DRAM. Per-layer tables for the edge
    gather are built with on-device AllGather of each core's rows.
  - Aggregation h = A @ x runs in a For_i hardware loop over 256-row dst
    windows: dma_gather of src rows (bf16) -> one-hot matrices on DVE ->
    TensorE matmuls accumulate window columns in f32 PSUM -> merged into
    SBUF-resident h^T (bf16).
  - Weight matmuls run node-tile-major: psum[128 nodes, feat_out] with
    lhsT = h^T (SBUF) / x^T (via transpose-mode dma_gather of the row-major
    DRAM activations, fetched lazily per 2048-node supertile); per-bucket
    weights; bias applied via a mask-row matmul so pad rows stay zero.
  - The fc3->lin1->lin2->out tail runs in transposed orientation per
    512-column piece, ending in outT [8, R] f32 per core.
  - Plan + program + jitted executable are cached per edge-index hash;
    repeat calls only repack inputs and rerun.
"""

import hashlib
import math
import os
import sys

sys.path.insert(0, "/opt/trn_rl_repo")

import numpy as np
import ml_dtypes

import concourse.bacc as bacc
import concourse.bass as bass
from concourse.bass import ds
import concourse.mybir as mybir
import concourse.tile as tile
from concourse import bass_utils

F32 = mybir.dt.float32
BF16 = mybir.dt.bfloat16
I16 = mybir.dt.int16
ACT = mybir.ActivationFunctionType
AOP = mybir.AluOpType
BF = ml_dtypes.bfloat16

NCORES = 8
P = 128
WIN = 256          # dst rows per aggregation window
MAX_DEG = 10
NB = MAX_DEG + 1
SLOPE = 0.01
GPIECE = 2048      # transpose-gather supertile (nodes)


def _ceil(a, b):
    return (a + b - 1) // b


# ---------------------------------------------------------------------------
# Host-side preprocessing (depends only on edge_index / N)
# ---------------------------------------------------------------------------

class Plan:
    pass


def _preprocess(N, src, dst):
    deg = np.bincount(dst, minlength=N).astype(np.int64)
    bucket = np.minimum(deg, MAX_DEG)

    order = np.argsort(bucket, kind="stable")
    pos = np.empty(N, np.int64)
    pos[order] = np.arange(N)
    core_of = pos % NCORES

    cnt_b = np.bincount(bucket, minlength=NB)
    seg_start = np.zeros(NB + 1, np.int64)
    seg_start[1:] = np.cumsum(cnt_b)
    S = _ceil(_ceil(cnt_b, NCORES), P) * P   # per-(core,bucket), 128-aligned
    off = np.zeros(NB + 1, np.int64)
    off[1:] = np.cumsum(S)
    R = int(math.ceil((off[NB] + 1) / WIN) * WIN)
    assert 2 * R <= 32767, f"block size {2*R} exceeds int16"

    local = off[bucket] + (pos - seg_start[bucket]) // NCORES
    new_global = core_of * R + local

    rows_old = np.full((NCORES, R), -1, np.int64)
    rows_old[core_of, local] = np.arange(N)

    # ---- edge slot streams ----
    BLK = 2 * R
    NBLK = 4
    W = R // WIN                       # windows per core
    ns = new_global[src]
    nd = new_global[dst]
    ecore = nd // R
    eldst = nd % R
    eblk = ns // BLK
    egrel = ns % BLK
    ewin = eldst // WIN

    # uniform L per (block, window) cell across cores:
    key = (ecore * NBLK + eblk) * W + ewin
    cnt = np.bincount(key, minlength=NCORES * NBLK * W).reshape(
        NCORES, NBLK, W)
    L = int(_ceil(max(1, int(cnt.max())), P) * P)
    M = NBLK * W * L

    # slot of edge e (in its core's stream): cell offset + rank within cell
    eorder = np.lexsort((ns, ewin, eblk, ecore))
    k2 = key[eorder]
    E = len(src)
    group_starts = np.flatnonzero(np.r_[True, k2[1:] != k2[:-1]])
    lens = np.diff(np.r_[group_starts, E])
    rank = np.arange(E) - np.repeat(group_starts, lens)
    cell = (eblk[eorder] * W + ewin[eorder])
    slot = cell * L + rank
    assert int(rank.max()) < L

    zrel = int(off[NB])                # relative zero/pad row inside a block
    gidx = np.full((NCORES, M), zrel, np.int16)
    dloc = np.zeros((NCORES, M), np.int16)
    ec = ecore[eorder]
    gidx[ec, slot] = egrel[eorder].astype(np.int16)
    dloc[ec, slot] = (eldst[eorder] % WIN).astype(np.int16)

    # wrapped idx stream [16, M/16]; dst window values [128, M/128] bf16
    gidx_w = np.ascontiguousarray(
        gidx.reshape(NCORES, M // 16, 16).transpose(0, 2, 1))
    dst_w = np.ascontiguousarray(
        dloc.reshape(NCORES, M // P, P).transpose(0, 2, 1)).astype(BF)

    # sequential idx for transpose-gathers [16, R/16]
    seq = np.arange(R, dtype=np.int16).reshape(R // 16, 16).T
    seq = np.ascontiguousarray(seq)

    # bucket of each 128-node tile (bucket sizes are 128-aligned)
    tiles = []
    for t in range(R // P):
        bkt = int(np.searchsorted(off[1:NB + 1], t * P, side="right"))
        tiles.append(min(bkt, NB - 1))

    plan = Plan()
    plan.N, plan.E, plan.R, plan.W, plan.M, plan.L = N, E, R, W, M, L
    plan.BLK, plan.NBLK = BLK, NBLK
    plan.S, plan.off = S, off
    plan.rows_old = rows_old
    plan.core_of, plan.local = core_of, local
    plan.gidx_w, plan.dst_w, plan.seq = gidx_w, dst_w, seq
    plan.tiles = tiles
    return plan


def _pad2(a, r, c):
    out = np.zeros((r, c), np.float32)
    out[: a.shape[0], : a.shape[1]] = a
    return out


def _pad3(a, n, r, c):
    out = np.zeros((n, r, c), np.float32)
    out[:, : a.shape[1], : a.shape[2]] = a
    return out


# ---------------------------------------------------------------------------
# Device program
# ---------------------------------------------------------------------------

def _build(plan):
    R, W, M, L = plan.R, plan.W, plan.M, plan.L
    BLK, NBLK = plan.BLK, plan.NBLK
    LC = L // P                         # gather chunks per cell
    NT = R // P                         # node tiles
    WSH = plan.WSH                      # weight-blob shard elems (bf16)

    nc = bacc.Bacc("TRN2", target_bir_lowering=False, debug=False,
                   num_devices=NCORES)

    def din(name, shape, dt):
        return nc.dram_tensor(name, shape, dt, kind="ExternalInput")

    def dint(name, shape, dt, shared=False):
        return nc.dram_tensor(name, shape, dt, kind="Internal",
                              addr_space="Shared" if shared else "Local")

    x_in = din("x_in", [R, P], BF16)
    gidx_t = din("gidx", [16, M // 16], I16)
    dstv_t = din("dstv", [P, M // P], BF16)
    seq_t = din("seq", [16, R // 16], I16)
    mask_t = din("mask", [1, R], BF16)
    iota_t = din("iota", [P, WIN], BF16)
    wsh_t = din("wsh", [1, WSH], BF16)

    w1l_t = din("w1l", [4, NB * P], BF16)
    w1r_t = din("w1r", [4, NB * P], BF16)
    b1_t = din("b1", [1, NB * P], BF16)
    fc1w_t = din("fc1w", [P, 256], BF16)
    fc1b_t = din("fc1b", [1, 256], BF16)
    b2_t = din("b2", [1, NB * 384], BF16)
    fc2w_t = din("fc2w", [384, 384], BF16)
    fc2b_t = din("fc2b", [1, 384], BF16)
    b3_t = din("b3", [1, NB * 384], BF16)
    fc3w_t = din("fc3w", [384, 256], BF16)
    fc3b_t = din("fc3b", [1, 256], BF16)
    l1w_t = din("l1w", [192, P], BF16)
    l1b_t = din("l1b", [1, P], BF16)
    l2w_t = din("l2w", [P, 64], BF16)
    l2b_t = din("l2b", [1, 64], BF16)
    ow_t = din("ow", [64, 8], BF16)
    ob_t = din("ob", [1, 8], BF16)

    outT_t = nc.dram_tensor("outT", [8, R], F32, kind="ExternalOutput")

    table1 = dint("table1", [NCORES * R, P], BF16, shared=True)
    x_loc = dint("x_loc", [R, P], BF16)
    wsh_loc = dint("wsh_loc", [1, WSH], BF16)
    wblob = dint("wblob", [NCORES, WSH], BF16, shared=True)
    c1_d = dint("c1", [R, P], BF16)
    ag1_d = dint("ag1", [R, 256], BF16)
    table2 = dint("table2", [NCORES * R, 256], BF16, shared=True)
    c2_d = dint("c2", [R, 384], BF16)
    ag2_d = dint("ag2", [R, 384], BF16)
    table3 = dint("table3", [NCORES * R, 384], BF16, shared=True)
    c3_d = dint("c3", [R, 384], BF16)

    groups = [list(range(NCORES))]
    wblob_f = wblob[:, :].rearrange("a b -> (a b)")
    STOP = int(os.environ.get("STOP_AFTER", "99"))

    class _StopBuild(Exception):
        pass

    import contextlib
    with tile.TileContext(nc) as tc:
        with contextlib.suppress(_StopBuild), \
             tc.tile_pool(name="persist", bufs=1) as pp:
            seq = pp.tile([P, R // 16], I16, tag="seq")
            for k in range(8):
                nc.sync.dma_start(seq[16 * k:16 * (k + 1), :], seq_t[:, :])
            mask = pp.tile([1, R], BF16, tag="mask")
            nc.sync.dma_start(mask[:], mask_t[:, :])
            iota = pp.tile([P, WIN], BF16, tag="iota")
            nc.sync.dma_start(iota[:], iota_t[:, :])

            nc.sync.dma_start(x_loc[:, :], x_in[:, :])
            nc.sync.dma_start(wsh_loc[:, :], wsh_t[:, :])
            nc.gpsimd.collective_compute(
                "AllGather", AOP.bypass, replica_groups=groups,
                ins=[x_loc[:, :]], outs=[table1[:, :]])
            nc.gpsimd.collective_compute(
                "AllGather", AOP.bypass, replica_groups=groups,
                ins=[wsh_loc[:, :]], outs=[wblob[:, :]])
            if STOP < 2:
                raise _StopBuild()

            # ---- helpers ----
            def aggregate(table, elem, hT, sp, pool, psp):
                """h^T (SBUF bf16 tiles, [128, R] each) += table[src] rows.
                For_i over W windows; static python over 4 src blocks."""
                nchunk = len(hT)
                for ht in hT:
                    nc.vector.memset(ht[:], 0.0)
                gidx = sp.tile([P, M // 16], I16, tag="gidx")
                for k in range(8):
                    nc.sync.dma_start(gidx[16 * k:16 * (k + 1), :],
                                      gidx_t[:, :])
                dstv = sp.tile([P, M // P], BF16, tag="dstv")
                nc.sync.dma_start(dstv[:], dstv_t[:, :])
                with tc.For_i(0, W, 1) as w:
                    g3s = []
                    for b in range(NBLK):
                        gt = pool.tile([P, LC * elem], BF16,
                                       tag=f"g{b % 2}", name=f"g{b}")
                        g3 = gt[:].rearrange("p (c e) -> p c e", e=elem)
                        col16 = b * W * (L // 16)
                        nc.gpsimd.dma_gather(
                            g3, table[b * BLK:(b + 1) * BLK, :],
                            gidx[:, ds(col16 + w * (L // 16), L // 16)],
                            L, L, elem, single_packet=False)
                        g3s.append(g3)
                    pss = [psp.tile([P, WIN], F32, space="PSUM",
                                    tag=f"ps{k}", name=f"ps{k}")
                           for k in range(nchunk)]
                    for b in range(NBLK):
                        for j in range(LC):
                            oh = pool.tile([P, WIN], BF16, tag=f"oh{b}")
                            nc.vector.tensor_tensor(
                                out=oh[:],
                                in0=dstv[:, ds(b * W * LC + w * LC + j, 1)]
                                .to_broadcast([P, WIN]),
                                in1=iota[:], op=AOP.is_equal)
                            for k in range(nchunk):
                                cw = min(P, elem - P * k)
                                nc.tensor.matmul(
                                    pss[k][:cw, :],
                                    lhsT=g3s[b][:, j, P * k:P * k + cw],
                                    rhs=oh[:],
                                    start=(b == 0 and j == 0),
                                    stop=(b == NBLK - 1 and j == LC - 1))
                    for k in range(nchunk):
                        cw = min(P, elem - P * k)
                        dap = hT[k][:cw, ds(w * WIN, WIN)]
                        nc.vector.tensor_tensor(
                            out=dap, in0=dap, in1=pss[k][:cw, :],
                            op=AOP.add)

            def gtr_piece(src_d, elem, pool, tag, s0, n):
                """transpose-gather rows [s0, s0+n) -> [128, elem/128, n]."""
                nch = elem // P
                t = pool.tile([P, nch * n], BF16, tag=tag)
                t3 = t[:].rearrange("p (c n) -> p c n", n=n)
                nc.gpsimd.dma_gather(
                    t3, src_d[:, :], seq[:, s0 // 16:(s0 + n) // 16],
                    n, n, elem, transpose=True, single_packet=False)
                return t3

            def supertiles(gp=GPIECE):
                for s0 in range(0, R, gp):
                    yield s0, min(gp, R - s0)

            if STOP < 3:
                raise _StopBuild()
            # ================= conv1 =================
            with tc.tile_pool(name="c1h", bufs=1) as hp, \
                 tc.tile_pool(name="c1s", bufs=1) as sp, \
                 tc.tile_pool(name="c1", bufs=2) as pool, \
                 tc.tile_pool(name="c1ps", bufs=2, space="PSUM") as psp:
                h1T = [hp.tile([P, R], BF16, tag="h1T", name="h1T")]
                if STOP >= 4:
                    aggregate(table1, P, h1T, sp, pool, psp)
                else:
                    nc.vector.memset(h1T[0][:], 0.0)
                if STOP < 5:
                    raise _StopBuild()
                w1l = sp.tile([4, NB * P], BF16, tag="w1l")
                nc.sync.dma_start(w1l[:], w1l_t[:, :])
                w1r = sp.tile([4, NB * P], BF16, tag="w1r")
                nc.sync.dma_start(w1r[:], w1r_t[:, :])
                b1 = sp.tile([1, NB * P], BF16, tag="b1")
                nc.sync.dma_start(b1[:], b1_t[:, :])
                for s0, n in supertiles():
                    xT = gtr_piece(x_in, P, pool, "x1T", s0, n)
                    for t in range(s0 // P, (s0 + n) // P):
                        j0 = t * P - s0
                        ps = psp.tile([P, P], F32, space="PSUM", tag="c1ps")
                        bkt = plan.tiles[t]
                        nc.tensor.matmul(
                            ps[:], lhsT=h1T[0][0:4, t * P:(t + 1) * P],
                            rhs=w1l[:, bkt * P:(bkt + 1) * P],
                            start=True, stop=False)
                        nc.tensor.matmul(
                            ps[:], lhsT=xT[0:4, 0, j0:j0 + P],
                            rhs=w1r[:, bkt * P:(bkt + 1) * P],
                            start=False, stop=False)
                        nc.tensor.matmul(
                            ps[:], lhsT=mask[0:1, t * P:(t + 1) * P],
                            rhs=b1[0:1, bkt * P:(bkt + 1) * P],
                            start=False, stop=True)
                        ot = pool.tile([P, P], BF16, tag="c1o")
                        nc.scalar.activation(ot[:], ps[:], ACT.Relu)
                        nc.sync.dma_start(c1_d[t * P:(t + 1) * P, :], ot[:])

            if STOP < 6:
                raise _StopBuild()
            # ================= fc1 =================
            with tc.tile_pool(name="f1", bufs=2) as pool, \
                 tc.tile_pool(name="f1ps", bufs=2, space="PSUM") as psp:
                fw = pool.tile([P, 256], BF16, tag="fc1w")
                nc.sync.dma_start(fw[:], fc1w_t[:, :])
                fb = pool.tile([1, 256], BF16, tag="fc1b")
                nc.sync.dma_start(fb[:], fc1b_t[:, :])
                for s0, n in supertiles():
                    cT = gtr_piece(c1_d, P, pool, "c1T", s0, n)
                    for t in range(s0 // P, (s0 + n) // P):
                        j0 = t * P - s0
                        ps = psp.tile([P, 256], F32, space="PSUM", tag="f1ps")
                        nc.tensor.matmul(ps[:], lhsT=cT[:, 0, j0:j0 + P],
                                         rhs=fw[:], start=True, stop=False)
                        nc.tensor.matmul(ps[:],
                                         lhsT=mask[0:1, t * P:(t + 1) * P],
                                         rhs=fb[:], start=False, stop=True)
                        ot = pool.tile([P, 256], BF16, tag="f1o")
                        nc.scalar.activation(ot[:], ps[:], ACT.Lrelu,
                                             alpha=SLOPE)
                        nc.sync.dma_start(ag1_d[t * P:(t + 1) * P, :], ot[:])
                nc.gpsimd.collective_compute(
                    "AllGather", AOP.bypass, replica_groups=groups,
                    ins=[ag1_d[:, :]], outs=[table2[:, :]])

            if STOP < 7:
                raise _StopBuild()
            # ================= conv2 =================
            with tc.tile_pool(name="c2h", bufs=1) as hp, \
                 tc.tile_pool(name="c2s", bufs=1) as sp, \
                 tc.tile_pool(name="c2", bufs=2) as pool, \
                 tc.tile_pool(name="c2ps", bufs=2, space="PSUM") as psp:
                h2T = [hp.tile([P, R], BF16, tag="h2T0", name="h2T0"),
                       hp.tile([P, R], BF16, tag="h2T1", name="h2T1")]
                aggregate(table2, 256, h2T, sp, pool, psp)
                b2 = sp.tile([1, NB * 384], BF16, tag="b2")
                nc.sync.dma_start(b2[:], b2_t[:, :])
                wt = {}
                cur_bkt = [-1]

                def w2load(bkt):
                    o = bkt * 2 * 192 * 384
                    for side in range(2):
                        for ki, kk in enumerate((P, 64)):
                            t_ = pool.tile([kk, 384], BF16,
                                           tag=f"w2_{side}_{ki}",
                                           name=f"w2_{side}_{ki}")
                            nc.sync.dma_start(
                                t_[:],
                                wblob_f[o:o + kk * 384].rearrange(
                                    "(a b) -> a b", b=384))
                            wt[(side, ki)] = t_
                            o += kk * 384
                    cur_bkt[0] = bkt

                for s0, n in supertiles():
                    xT = gtr_piece(ag1_d, 256, pool, "x2T", s0, n)
                    for t in range(s0 // P, (s0 + n) // P):
                        j0 = t * P - s0
                        if plan.tiles[t] != cur_bkt[0]:
                            w2load(plan.tiles[t])
                        ps = psp.tile([P, 384], F32, space="PSUM", tag="c2ps")
                        bkt = plan.tiles[t]
                        for ki, kk in enumerate((P, 64)):
                            nc.tensor.matmul(
                                ps[:],
                                lhsT=h2T[ki][:kk, t * P:(t + 1) * P],
                                rhs=wt[(0, ki)][:],
                                start=(ki == 0), stop=False)
                            nc.tensor.matmul(
                                ps[:], lhsT=xT[:kk, ki, j0:j0 + P],
                                rhs=wt[(1, ki)][:],
                                start=False, stop=False)
                        nc.tensor.matmul(
                            ps[:], lhsT=mask[0:1, t * P:(t + 1) * P],
                            rhs=b2[0:1, bkt * 384:(bkt + 1) * 384],
                            start=False, stop=True)
                        ot = pool.tile([P, 384], BF16, tag="c2o")
                        nc.scalar.activation(ot[:], ps[:], ACT.Relu)
                        nc.sync.dma_start(c2_d[t * P:(t + 1) * P, :], ot[:])

            if STOP < 8:
                raise _StopBuild()
            # ================= fc2 =================
            with tc.tile_pool(name="f2", bufs=2) as pool, \
                 tc.tile_pool(name="f2ps", bufs=2, space="PSUM") as psp:
                fws = []
                for ki, (p0, kk) in enumerate(((0, P), (P, P), (2 * P, 32))):
                    t_ = pool.tile([kk, 384], BF16, tag=f"fc2w{ki}")
                    nc.sync.dma_start(t_[:], fc2w_t[p0:p0 + kk, :])
                    fws.append(t_)
                fb = pool.tile([1, 384], BF16, tag="fc2b")
                nc.sync.dma_start(fb[:], fc2b_t[:, :])
                for s0, n in supertiles(1024):
                    cT = gtr_piece(c2_d, 384, pool, "c2T", s0, n)
                    for t in range(s0 // P, (s0 + n) // P):
                        j0 = t * P - s0
                        ps = psp.tile([P, 384], F32, space="PSUM", tag="f2ps")
                        for ki, (c, kk) in enumerate(((0, P), (1, P),
                                                      (2, 32))):
                            nc.tensor.matmul(
                                ps[:], lhsT=cT[:kk, c, j0:j0 + P],
                                rhs=fws[ki][:], start=(ki == 0), stop=False)
                        nc.tensor.matmul(ps[:],
                                         lhsT=mask[0:1, t * P:(t + 1) * P],
                                         rhs=fb[:], start=False, stop=True)
                        ot = pool.tile([P, 384], BF16, tag="f2o")
                        nc.scalar.activation(ot[:], ps[:], ACT.Lrelu,
                                             alpha=SLOPE)
                        nc.sync.dma_start(ag2_d[t * P:(t + 1) * P, :], ot[:])
                nc.gpsimd.collective_compute(
                    "AllGather", AOP.bypass, replica_groups=groups,
                    ins=[ag2_d[:, :]], outs=[table3[:, :]])

            if STOP < 9:
                raise _StopBuild()
            # ================= conv3 =================
            with tc.tile_pool(name="c3h", bufs=1) as hp, \
                 tc.tile_pool(name="c3s", bufs=1) as sp, \
                 tc.tile_pool(name="c3", bufs=2) as pool, \
                 tc.tile_pool(name="c3ps", bufs=2, space="PSUM") as psp:
                h3T = [hp.tile([P, R], BF16, tag=f"h3T{k}", name=f"h3T{k}")
                       for k in range(3)]
                aggregate(table3, 384, h3T, sp, pool, psp)
                b3 = sp.tile([1, NB * 384], BF16, tag="b3")
                nc.sync.dma_start(b3[:], b3_t[:, :])
                wt = {}
                cur_bkt = [-1]

                def w3load(bkt):
                    o = plan.W3OFF + bkt * 2 * 3 * P * 384
                    for side in range(2):
                        for ki in range(3):
                            t_ = pool.tile([P, 384], BF16,
                                           tag=f"w3_{side}_{ki}",
                                           name=f"w3_{side}_{ki}")
                            nc.sync.dma_start(
                                t_[:],
                                wblob_f[o:o + P * 384].rearrange(
                                    "(a b) -> a b", b=384))
                            wt[(side, ki)] = t_
                            o += P * 384
                    cur_bkt[0] = bkt

                for s0, n in supertiles(1024):
                    xT = gtr_piece(ag2_d, 384, pool, "x3T", s0, n)
                    for t in range(s0 // P, (s0 + n) // P):
                        j0 = t * P - s0
                        if plan.tiles[t] != cur_bkt[0]:
                            w3load(plan.tiles[t])
                        ps = psp.tile([P, 384], F32, space="PSUM", tag="c3ps")
                        bkt = plan.tiles[t]
                        for ki in range(3):
                            nc.tensor.matmul(
                                ps[:],
                                lhsT=h3T[ki][:, t * P:(t + 1) * P],
                                rhs=wt[(0, ki)][:],
                                start=(ki == 0), stop=False)
                            nc.tensor.matmul(
                                ps[:], lhsT=xT[:, ki, j0:j0 + P],
                                rhs=wt[(1, ki)][:],
                                start=False, stop=False)
                        nc.tensor.matmul(
                            ps[:], lhsT=mask[0:1, t * P:(t + 1) * P],
                            rhs=b3[0:1, bkt * 384:(bkt + 1) * 384],
                            start=False, stop=True)
                        ot = pool.tile([P, 384], BF16, tag="c3o")
                        nc.scalar.activation(ot[:], ps[:], ACT.Relu)
                        nc.sync.dma_start(c3_d[t * P:(t + 1) * P, :], ot[:])

            if STOP < 10:
                raise _StopBuild()
            # ====== tail: fc3 -> lin1 -> lin2 -> out (transposed) ======
            with tc.tile_pool(name="tl", bufs=2) as pool, \
                 tc.tile_pool(name="tlps", bufs=1, space="PSUM") as psp:
                f3w = []
                for ki, (p0, kk) in enumerate(((0, P), (P, P), (2 * P, 32))):
                    t_ = pool.tile([kk, 256], BF16, tag=f"fc3w{ki}")
                    nc.sync.dma_start(t_[:], fc3w_t[p0:p0 + kk, :])
                    f3w.append(t_)
                f3b = pool.tile([1, 256], BF16, tag="fc3b")
                nc.sync.dma_start(f3b[:], fc3b_t[:, :])
                w1 = []
                for ki, (p0, kk) in enumerate(((0, P), (P, 64))):
                    t_ = pool.tile([kk, P], BF16, tag=f"l1w{ki}")
                    nc.sync.dma_start(t_[:], l1w_t[p0:p0 + kk, :])
                    w1.append(t_)
                b1r = pool.tile([1, P], BF16, tag="l1b")
                nc.sync.dma_start(b1r[:], l1b_t[:, :])
                w2 = pool.tile([P, 64], BF16, tag="l2w")
                nc.sync.dma_start(w2[:], l2w_t[:, :])
                b2r = pool.tile([1, 64], BF16, tag="l2b")
                nc.sync.dma_start(b2r[:], l2b_t[:, :])
                wo = pool.tile([64, 8], BF16, tag="ow")
                nc.sync.dma_start(wo[:], ow_t[:, :])
                bo = pool.tile([1, 8], BF16, tag="ob")
                nc.sync.dma_start(bo[:], ob_t[:, :])
                CP = 512
                for s0, n in supertiles(1024):
                    cT = gtr_piece(c3_d, 384, pool, "c3T", s0, n)
                    for c0 in range(s0, s0 + n, CP):
                        cw = min(CP, s0 + n - c0)
                        j0 = c0 - s0
                        msl = mask[0:1, c0:c0 + cw]
                        f3o = []
                        for ko, (o0, oc) in enumerate(((0, P), (P, 64))):
                            ps = psp.tile([oc, CP], F32, space="PSUM",
                                          tag=f"f3ps{ko}")
                            for ki, (c, kk) in enumerate(((0, P), (1, P),
                                                          (2, 32))):
                                nc.tensor.matmul(
                                    ps[:, :cw],
                                    lhsT=f3w[ki][:kk, o0:o0 + oc],
                                    rhs=cT[:kk, c, j0:j0 + cw],
                                    start=(ki == 0), stop=False)
                            nc.tensor.matmul(ps[:, :cw],
                                             lhsT=f3b[0:1, o0:o0 + oc],
                                             rhs=msl, start=False, stop=True)
                            ot = pool.tile([oc, CP], BF16, tag=f"f3o{ko}")
                            nc.scalar.activation(ot[:, :cw], ps[:, :cw],
                                                 ACT.Lrelu, alpha=SLOPE)
                            f3o.append(ot)
                        ps1 = psp.tile([P, CP], F32, space="PSUM", tag="l1ps")
                        for ki, kk in enumerate((P, 64)):
                            nc.tensor.matmul(ps1[:, :cw], lhsT=w1[ki][:],
                                             rhs=f3o[ki][:kk, :cw],
                                             start=(ki == 0), stop=False)
                        nc.tensor.matmul(ps1[:, :cw], lhsT=b1r[:],
                                         rhs=msl, start=False, stop=True)
                        l1o = pool.tile([P, CP], BF16, tag="l1o")
                        nc.scalar.activation(l1o[:, :cw], ps1[:, :cw],
                                             ACT.Copy)
                        ps2 = psp.tile([64, CP], F32, space="PSUM",
                                       tag="l2ps")
                        nc.tensor.matmul(ps2[:, :cw], lhsT=w2[:],
                                         rhs=l1o[:, :cw],
                                         start=True, stop=False)
                        nc.tensor.matmul(ps2[:, :cw], lhsT=b2r[:],
                                         rhs=msl, start=False, stop=True)
                        l2o = pool.tile([64, CP], BF16, tag="l2o")
                        nc.scalar.activation(l2o[:, :cw], ps2[:, :cw],
                                             ACT.Copy)
                        ps3 = psp.tile([8, CP], F32, space="PSUM", tag="ops")
                        nc.tensor.matmul(ps3[:, :cw], lhsT=wo[:],
                                         rhs=l2o[:, :cw],
                                         start=True, stop=False)
                        nc.tensor.matmul(ps3[:, :cw], lhsT=bo[:],
                                         rhs=msl, start=False, stop=True)
                        oo = pool.tile([8, CP], F32, tag="oo")
                        nc.scalar.activation(oo[:, :cw], ps3[:, :cw],
                                             ACT.Sigmoid)
                        nc.sync.dma_start(outT_t[:, c0:c0 + cw], oo[:, :cw])

    nc.compile()
    return nc


# ---------------------------------------------------------------------------
# Input packing
# ---------------------------------------------------------------------------

def _pack_inputs(plan, x, wd):
    R, M = plan.R, plan.M

    x_in = np.zeros((NCORES, R, P), BF)
    x_in[plan.core_of, plan.local, 0:3] = x.astype(BF)

    mask = np.zeros((NCORES, 1, R), BF)
    mask[plan.core_of, 0, plan.local] = 1.0

    iota = np.tile(np.arange(WIN, dtype=np.float32), (P, 1)).astype(BF)

    # weight blob (bf16): conv2 tiles then conv3 tiles
    blob = []
    w2l = _pad3(wd["Wl2"], NB, 192, 384).astype(BF)
    w2r = _pad3(wd["Wr2"], NB, 192, 384).astype(BF)
    for bkt in range(NB):
        for wmat in (w2l, w2r):
            for p0, kk in ((0, P), (P, 64)):
                blob.append(wmat[bkt, p0:p0 + kk, :].ravel())
    w3off = sum(b.size for b in blob)
    w3l = _pad3(wd["Wl3"], NB, 384, 384).astype(BF)
    w3r = _pad3(wd["Wr3"], NB, 384, 384).astype(BF)
    for bkt in range(NB):
        for wmat in (w3l, w3r):
            for ki in range(3):
                blob.append(wmat[bkt, P * ki:P * (ki + 1), :].ravel())
    blob = np.concatenate(blob)
    WSH = _ceil(len(blob), NCORES)
    blobp = np.zeros(NCORES * WSH, BF)
    blobp[: len(blob)] = blob
    plan.WSH = WSH
    plan.W3OFF = w3off

    def b16(a):
        return np.ascontiguousarray(a).astype(BF)

    common = {
        "iota": iota,
        "seq": plan.seq,
        "w1l": b16(_pad3(wd["Wl1"], NB, 4, P).transpose(1, 0, 2)
                   .reshape(4, NB * P)),
        "w1r": b16(_pad3(wd["Wr1"], NB, 4, P).transpose(1, 0, 2)
                   .reshape(4, NB * P)),
        "b1": b16(_pad2(wd["bl1"], NB, P).reshape(1, NB * P)),
        "fc1w": b16(_pad2(wd["fc1W"], P, 256)),
        "fc1b": b16(_pad2(wd["fc1b"][None, :], 1, 256)),
        "b2": b16(_pad2(wd["bl2"], NB, 384).reshape(1, NB * 384)),
        "fc2w": b16(_pad2(wd["fc2W"], 384, 384)),
        "fc2b": b16(_pad2(wd["fc2b"][None, :], 1, 384)),
        "b3": b16(_pad2(wd["bl3"], NB, 384).reshape(1, NB * 384)),
        "fc3w": b16(_pad2(wd["fc3W"], 384, 256)),
        "fc3b": b16(_pad2(wd["fc3b"][None, :], 1, 256)),
        "l1w": b16(_pad2(wd["lin1W"], 192, P)),
        "l1b": b16(_pad2(wd["lin1b"][None, :], 1, P)),
        "l2w": b16(_pad2(wd["lin2W"], P, 64)),
        "l2b": b16(_pad2(wd["lin2b"][None, :], 1, 64)),
        "ow": b16(_pad2(wd["outW"], 64, 8)),
        "ob": b16(_pad2(wd["outb"][None, :], 1, 8)),
    }
    in_maps = []
    for c in range(NCORES):
        m = dict(common)
        m["x_in"] = x_in[c]
        m["mask"] = mask[c]
        m["gidx"] = plan.gidx_w[c]
        m["dstv"] = plan.dst_w[c]
        m["wsh"] = blobp[c * WSH:(c + 1) * WSH][None, :]
        in_maps.append(m)
    return in_maps


# ---------------------------------------------------------------------------
# kernel entry (with per-edge-hash caching of plan + program + jit)
# ---------------------------------------------------------------------------

_CACHE = {}

WEIGHT_KEYS = ["Wl1", "Wr1", "bl1", "fc1W", "fc1b", "Wl2", "Wr2", "bl2",
               "fc2W", "fc2b", "Wl3", "Wr3", "bl3", "fc3W", "fc3b",
               "lin1W", "lin1b", "lin2W", "lin2b", "outW", "outb"]


def _make_runner(nc):
    """Cacheable jitted runner for nc (adapted from bass2jax PJRT path)."""
    import jax
    from jax.sharding import Mesh, PartitionSpec
    from jax.experimental.shard_map import shard_map
    from concourse import bass2jax

    bass2jax.install_neuronx_cc_hook()
    partition_name = (nc.partition_id_tensor.name
                      if nc.partition_id_tensor else None)
    in_names, out_names, out_avals, zero_shapes = [], [], [], []
    for alloc in nc.m.functions[0].allocations:
        if not isinstance(alloc, mybir.MemoryLocationSet):
            continue
        name = alloc.memorylocations[0].name
        if alloc.kind == "ExternalInput":
            if name != partition_name:
                in_names.append(name)
        elif alloc.kind == "ExternalOutput":
            out_names.append(name)
            shape = tuple(alloc.tensor_shape)
            dtype = mybir.dt.np(alloc.dtype)
            out_avals.append(jax.core.ShapedArray(shape, dtype))
            zero_shapes.append((shape, dtype))
    n_params = len(in_names)
    n_outs = len(out_avals)
    in_names_all = list(in_names) + out_names + (
        [partition_name] if partition_name else [])
    donate = tuple(range(n_params, n_params + n_outs))

    def _body(*args):
        operands = list(args)
        if partition_name is not None:
            operands.append(bass2jax.partition_id_tensor())
        outs = bass2jax._bass_exec_p.bind(
            *operands, out_avals=tuple(out_avals),
            in_names=tuple(in_names_all), out_names=tuple(out_names),
            lowering_input_output_aliases=(), sim_require_finite=True,
            sim_require_nnan=True, nc=nc)
        return tuple(outs)

    devices = jax.devices()[:NCORES]
    mesh = Mesh(np.asarray(devices), ("core",))
    in_specs = (PartitionSpec("core"),) * (n_params + n_outs)
    out_specs = (PartitionSpec("core"),) * len(out_names)
    sharded = jax.jit(
        shard_map(_body, mesh=mesh, in_specs=in_specs,
                  out_specs=out_specs, check_rep=False),
        donate_argnums=donate, keep_unused=True)

    def run(in_maps):
        concat_in = [
            np.concatenate([np.asarray(m[name]) for m in in_maps], axis=0)
            for name in in_names]
        concat_zeros = [
            np.zeros((NCORES * s[0], *s[1:]), d) for (s, d) in zero_shapes]
        outs = sharded(*concat_in, *concat_zeros)
        return [
            {name: np.asarray(outs[i]).reshape(NCORES, *out_avals[i].shape)[c]
             for i, name in enumerate(out_names)}
            for c in range(NCORES)]

    return run


def kernel(**inputs):
    x = np.ascontiguousarray(np.asarray(inputs["x"], dtype=np.float32))
    edge_index = np.ascontiguousarray(
        np.asarray(inputs["edge_index"], dtype=np.int64))
    N = x.shape[0]

    ekey = (hashlib.blake2b(edge_index.tobytes(), digest_size=16).hexdigest(),
            N)

    wd = {k: np.asarray(inputs[k], np.float32) for k in WEIGHT_KEYS}

    if ekey in _CACHE:
        plan, run = _CACHE[ekey]
        in_maps = _pack_inputs(plan, x, wd)
    else:
        plan = _preprocess(N, np.asarray(edge_index[0]),
                           np.asarray(edge_index[1]))
        in_maps = _pack_inputs(plan, x, wd)   # sets plan.WSH / plan.W3OFF
        nc = _build(plan)
        run = _make_runner(nc)
        _CACHE[ekey] = (plan, run)

    results = run(in_maps)
    kernel._last_results = None

    out = np.empty((N, 6), np.float32)
    for c in range(NCORES):
        oT = np.asarray(results[c]["outT"])   # [8, R]
        rows = plan.rows_old[c]
        valid = rows >= 0
        out[rows[valid]] = oT[:6, valid].T
    return out
